# revision 20
# baseline (speedup 1.0000x reference)
"""Trainium2 Bass kernel for nn_BilinearPolicy (dense_mlp).

Math (reference):
  ob = trunk_obs(obs)      : [B,256] -> 2048 -> 2048 -> 2048 -> 16384 (ReLU between)
  dl = trunk_dlt(deltas)   : same shapes, different weights
  pred[b,a] = sum_f ob[b, a*512+f] * dl[b, f*32+a]            : [B, 32]

Strategy:
  * Data-parallel over batch: 8 cores x 512 rows, zero collectives.
  * Feature-major activations on chip ([feat(part), batch(free)]), so the
    torch-layout weights [din, dout] are used directly as matmul lhsT tiles
    and no transposes are ever needed. Inputs are transposed on host.
  * fp16 matmuls with fp32 PSUM accumulation (same PE rate as bf16,
    8x finer mantissa). Biases applied during the
    mandatory PSUM->SBUF eviction on the Scalar engine (Relu / Identity).
  * Mixed-precision L3 (79% of the FLOPs): L2's output features are
    permuted on host (free: permute W2 columns + W3 rows) so the Q coldest
    128-feature blocks (host-calibrated power on a 256-row subsample) come
    first; those blocks are quantized to fp8-e4m3 and contracted with
    DoubleRow matmuls (2 k-blocks per MM at ~1.07x a normal MM, i.e.
    ~1.9x throughput on those blocks). The L3 PSUM
    runs in G*y units (fp16 W3 tiles scaled by G on host; fp8 products
    carry G via the weight scale G/s against the activation scale s);
    evictions apply 1/G before the bias.
  * dl's last-layer weight columns are permuted on host from (f,a) to (a,f)
    ordering, so the bilinear diagonal becomes an elementwise multiply of
    the two trunk outputs + segmented reduction. The z path (evict, mul,
    accumulate) runs in fp16 (2x DVE rate); one one-hot mask matmul per
    action accumulates pred^T [32, 512] in a single PSUM tile.
  * Weight streaming: weights pre-grouped on host so each DMA moves large
    contiguous partition lines, round-robin over the sync + scalar HWDGE
    queues. Small constants go via gpsimd SWDGE.
"""

from contextlib import ExitStack

import numpy as np
import ml_dtypes

B, OBS, H, F, A = 4096, 256, 2048, 512, 32
DOUT = F * A            # 16384
NCORES = 8
BPC = B // NCORES       # 512 batch rows per core
P = 128

KT = [2, 16, 16, 16]    # k-tiles per layer
MT = [16, 16, 16, 128]  # m-tiles per layer
GR = [8, 4, 4, 4]       # m-tiles grouped per weight DMA

Q = 8                   # L3 k-blocks (of 16) quantized to fp8-DoubleRow
NB = 16 - Q             # L3 k-blocks kept in fp16
G = 4096.0              # L3 PSUM scale
CAL_ROWS = 256          # host calibration subsample
HMARGIN = 120.0         # fp8 |h*s| target max (overflow at 240)

F16 = np.float16
E4M3 = ml_dtypes.float8_e4m3   # IEEE-style: max 240, matches TRN FP8_EXP4

# Filled with the BassKernelResults of the most recent run (for test harness).
LAST_RESULTS = None


def _tile_weight(w, G_):
    """[D1, D2] fp32 -> [D2/(128G), 128(k), G*D1] bf16. Slice
    [:, (g*Kt + j)*128 : +128] of group tile mtg is the lhsT for
    k-tile j of m-tile mtg*G+g; every partition line is contiguous."""
    d1, d2 = w.shape
    kt, mt = d1 // P, d2 // P
    wt = w.reshape(kt, P, mt, P).transpose(2, 1, 0, 3)      # [mt, k, j, m]
    wt = wt.reshape(mt // G_, G_, P, kt * P).transpose(0, 2, 1, 3)
    wt = wt.reshape(mt // G_, P, G_ * kt * P)
    return np.ascontiguousarray(wt.astype(F16))


def _tile_bias(b):
    """[D2] fp32 -> [128, D2/128] fp32; column mt holds bias for m-tile mt
    as a per-partition scalar."""
    return np.ascontiguousarray(b.reshape(-1, P).T.astype(np.float32))


def _build_program():
    import concourse.bass as bass
    import concourse.tile as tile
    from concourse import bacc, bass_isa, mybir
    from concourse.bass import ts

    dt = mybir.dt
    AF = mybir.ActivationFunctionType
    DRM = mybir.MatmulPerfMode.DoubleRow

    nc = bacc.Bacc(
        "TRN2",
        target_bir_lowering=False,
        debug=False,
        enable_asserts=True,
        num_devices=NCORES,
    )

    def din(name, shape, dtype):
        return nc.dram_tensor(name, shape, dtype, kind="ExternalInput").ap()

    x_d = {
        "o": din("xo", [P, 2 * BPC], dt.float16),
        "d": din("xd", [P, 2 * BPC], dt.float16),
    }
    w_d = {}
    b_d = {}
    for t in ("o", "d"):
        for l in range(3):
            w_d[t, l] = din(f"{t}w{l}", [MT[l] // GR[l], P, GR[l] * KT[l] * P],
                            dt.float16)
        # L3 split: fp16 part (NB k-blocks) + fp8 DoubleRow part (Q k-blocks)
        w_d[t, "3b"] = din(f"{t}w3b", [A, P, GR[3] * NB * P], dt.float16)
        w_d[t, "3q"] = din(f"{t}w3q", [A, P, Q * GR[3] * P], dt.float8e4)
        for l in range(4):
            b_d[t, l] = din(f"{t}b{l}", [P, MT[l]], dt.float32)
    scales_d = din("scales", [P, 2], dt.float32)
    pred_d = nc.dram_tensor("pred", [A, BPC], dt.float16, kind="ExternalOutput").ap()

    with tile.TileContext(nc) as tc, ExitStack() as ctx:
        const = ctx.enter_context(tc.tile_pool(name="const", bufs=1))
        wp = ctx.enter_context(tc.tile_pool(name="wp", bufs=4))
        wp3 = ctx.enter_context(tc.tile_pool(name="wp3", bufs=5))
        act = ctx.enter_context(tc.tile_pool(name="act", bufs=1))
        ev = ctx.enter_context(tc.tile_pool(name="ev", bufs=3))
        ps = ctx.enter_context(tc.tile_pool(name="ps", bufs=8, space="PSUM"))

        # weight DMAs round-robin over the sync + scalar HWDGE queues so
        # supply isn't capped by a single queue's descriptor rate
        dma_engs = [nc.sync, nc.scalar]
        rr = [0]

        def wdma(dst, src):
            dma_engs[rr[0] % len(dma_engs)].dma_start(dst, src)
            rr[0] += 1

        # inputs + L0 weights first, split across both HWDGE queues in
        # first-use order so the PE can start within a few us; other small
        # constants go on the gpsimd SWDGE queue
        x_sb = {}
        w0_tiles = {}
        for t in ("o", "d"):
            x_sb[t] = const.tile([P, 2, BPC], dt.float16,
                                 tag=f"x{t}", name=f"x{t}")
            xsrc = x_d[t].rearrange("p (k n) -> p k n", n=BPC)
            for k in range(2):
                # interleave x halves with L0 weight groups so the first
                # matmul's rhs and lhsT stream on different queues
                wdma(x_sb[t][:, k, :], xsrc[:, k, :])
                w0_tiles[t, k] = wp.tile([P, GR[0] * KT[0] * P], dt.float16,
                                         tag="w0", name="w0t", bufs=4)
                wdma(w0_tiles[t, k][:], w_d[t, 0][k])
        bias_sb = {}
        for t in ("o", "d"):
            for l in range(4):
                bias_sb[t, l] = const.tile([P, MT[l]], dt.float32,
                                           tag=f"b{t}{l}", name=f"b{t}{l}")
                nc.gpsimd.dma_start(bias_sb[t, l][:], b_d[t, l][:])
        scales_sb = const.tile([P, 2], dt.float32, tag="scales")
        nc.gpsimd.dma_start(scales_sb[:], scales_d[:])
        scol = {"o": 0, "d": 1}

        # ---- Trunks: layers 0..2 with ReLU, feature-major throughout.
        # The two trunks are interleaved layer-by-layer so the PE has twice
        # the work per phase start, covering the weight-stream warm-up.
        # L2's output features are host-permuted: blocks 0..Q-1 (coldest)
        # are evicted to fp8 with scale s_t, blocks Q..15 to fp16.
        cur = dict(x_sb)
        hq_sb = {}
        for l in range(3):
            for t in ("o", "d"):
                out_t = act.tile([P, MT[l], BPC], dt.float16,
                                 tag=f"h{t}{l % 2}", name=f"h{t}{l}")
                if l == 2:
                    hq_sb[t] = act.tile([P, Q, BPC], dt.float8e4,
                                        tag=f"hq{t}", name=f"hq{t}")
                for mtg in range(MT[l] // GR[l]):
                    if l == 0:
                        wt = w0_tiles[t, mtg]
                    else:
                        wt = wp.tile([P, GR[l] * KT[l] * P], dt.float16,
                                     tag="wbig", name="wt", bufs=3)
                        if l == 1 and mtg == 0:
                            # first L1 group rides the idle gpsimd SWDGE
                            # queue to absorb the warmup bandwidth deficit
                            nc.gpsimd.dma_start(wt[:], w_d[t, l][mtg])
                        else:
                            wdma(wt[:], w_d[t, l][mtg])
                    for g in range(GR[l]):
                        mt = mtg * GR[l] + g
                        wcol = g
                        pt = ps.tile([P, BPC], dt.float32, tag="mm")
                        for j in range(KT[l]):
                            nc.tensor.matmul(
                                pt[:], wt[:, ts(wcol * KT[l] + j, P)],
                                cur[t][:, j, :],
                                start=(j == 0), stop=(j == KT[l] - 1),
                            )
                        if l == 2 and mt < Q:
                            # Relu(psum*s + s*b) = s*Relu(psum+b) -> fp8
                            nc.scalar.activation(
                                hq_sb[t][:, mt, :], pt[:], AF.Relu,
                                bias=bias_sb[t, 2][:, mt:mt + 1],
                                scale=scales_sb[:, scol[t]:scol[t] + 1],
                            )
                        else:
                            slot = mt if l < 2 else mt - Q
                            nc.scalar.activation(
                                out_t[:, slot, :], pt[:], AF.Relu,
                                bias=bias_sb[t, l][:, mt:mt + 1],
                            )
                cur[t] = out_t
        h = cur

        # interleaved L3 MM sequence: every DoubleRow LDWEIGHTS (no FWL,
        # 256 cols) hides under a long predecessor stream
        seq = []
        for j in range(max(NB, Q // 2)):
            if j < Q // 2:
                seq.append(("q", j))
            if j < NB:
                seq.append(("b", j))

        # ---- Layer 3 + bilinear diagonal, fused per 128-feature tile.
        for a in range(A):  # one weight DMA pair per trunk covers the action
            wtb = {}
            wtq = {}
            for t in ("o", "d"):
                wtb[t] = wp3.tile([P, GR[3] * NB * P], dt.float16,
                                  tag="w3b", name=f"w3b{t}")
                wdma(wtb[t][:], w_d[t, "3b"][a])
                wtq[t] = wp3.tile([P, Q, GR[3] * P], dt.float8e4,
                                  tag="w3q", name=f"w3q{t}", bufs=4)
                nc.gpsimd.dma_start(wtq[t][:], w_d[t, "3q"][a].rearrange(
                    "p (q m) -> p q m", q=Q))
            zt = {}
            for g in range(GR[3]):
                mt = a * 4 + g
                s = {}
                for t in ("o", "d"):
                    pt = ps.tile([P, BPC], dt.float32, tag="mm")
                    for i, (kind, j) in enumerate(seq):
                        if kind == "b":
                            nc.tensor.matmul(
                                pt[:], wtb[t][:, ts(g * NB + j, P)],
                                h[t][:, j, :],
                                start=(i == 0), stop=(i == len(seq) - 1),
                            )
                        else:
                            nc.tensor.matmul(
                                pt[:], wtq[t][:, 2 * j:2 * j + 2, ts(g, P)],
                                hq_sb[t][:, 2 * j:2 * j + 2, :],
                                start=(i == 0), stop=(i == len(seq) - 1),
                                perf_mode=DRM,
                            )
                    s[t] = ev.tile([P, BPC], dt.float16, tag="evict",
                                   name=f"s{t}", bufs=2)
                    nc.scalar.activation(
                        s[t][:], pt[:], AF.Identity,
                        bias=bias_sb[t, 3][:, mt:mt + 1],
                        scale=1.0 / G,
                    )
                zt[g] = ev.tile([P, BPC], dt.float16, tag=f"z{g % 2}",
                                name=f"z{g}", bufs=2)
                nc.vector.tensor_mul(zt[g][:], s["o"][:], s["d"][:])
                if g >= 1:
                    nc.vector.tensor_add(zt[0][:], zt[0][:], zt[g][:])
            # pred[a,:] via gpsimd cross-partition reduce (frees the PE of
            # 32 mask matmuls); every partition of the scratch holds the
            # sum, so row `a` DMAs straight out
            prsum = ev.tile([P, BPC], dt.float16, tag="prsum", bufs=2)
            nc.gpsimd.partition_all_reduce(
                prsum[:], zt[0][:], channels=P,
                reduce_op=bass_isa.ReduceOp.add)
            nc.sync.dma_start(pred_d[a:a + 1, :], prsum[a:a + 1, :])

    nc.compile()
    return nc


def _calibrate(inputs):
    """Host calibration: per-trunk L2-output feature power + max on a
    CAL_ROWS-row subsample. Returns per-trunk (perm, s)."""
    out = {}
    for t, pfx, xk in (("o", "obs", "obs"), ("d", "dlt", "deltas")):
        hc = np.asarray(inputs[xk][:CAL_ROWS], np.float32)
        for l in range(3):
            W = np.asarray(inputs[f"{pfx}_W{l}"], np.float32)
            b = np.asarray(inputs[f"{pfx}_b{l}"], np.float32)
            hc = np.maximum(hc @ W + b, 0.0)
        power = (hc ** 2).mean(0)
        perm = np.argsort(power)
        hmax = hc[:, perm[:Q * P]].max()
        out[t] = (perm, HMARGIN / max(hmax, 1e-9))
    return out


def _prep_inputs(inputs):
    """Host-side layout/dtype prep shared across cores + per-core slices."""
    shared = {}
    cal = _calibrate(inputs)

    for t, pfx in (("o", "obs"), ("d", "dlt")):
        perm, s = cal[t]
        for l in range(3):
            w = np.asarray(inputs[f"{pfx}_W{l}"], np.float32)
            b = np.asarray(inputs[f"{pfx}_b{l}"], np.float32)
            if l == 2:
                w = w[:, perm]
                b = b[perm]
            shared[f"{t}w{l}"] = _tile_weight(w, GR[l])
            bt = _tile_bias(b)
            if l == 2:
                bt[:, :Q] *= s  # fp8 evictions get scale-folded biases
            shared[f"{t}b{l}"] = bt

        w3 = np.asarray(inputs[f"{pfx}_W3"], np.float32)
        b3 = np.asarray(inputs[f"{pfx}_b3"], np.float32)
        if t == "d":
            # permute columns (f,a) -> (a,f) to match obs layout
            w3 = w3.reshape(H, F, A).transpose(0, 2, 1).reshape(H, DOUT)
            b3 = b3.reshape(F, A).T.reshape(DOUT)
        w3 = w3[perm, :]
        # fp16 part: k-blocks Q..15, scaled by G; column order within an
        # action must be (g*NB + j)*128 + m
        w3b = w3[Q * P:, :] * G                         # [NB*128, 16384]
        w3b = w3b.reshape(NB, P, A, GR[3], P)           # [j, k, a, g, m]
        w3b = w3b.transpose(2, 1, 3, 0, 4)              # [a, k, g, j, m]
        w3b = w3b.reshape(A, P, GR[3] * NB * P)
        shared[f"{t}w3b"] = np.ascontiguousarray(w3b.astype(F16))
        # fp8 part: k-blocks 0..Q-1, scaled by G/s; layout [A, P, Q, 4*128]
        w3q = w3[:Q * P, :] * (G / s)                   # [Q*128, 16384]
        assert np.abs(w3q).max() < 240.0, np.abs(w3q).max()
        w3q = w3q.reshape(Q, P, A, GR[3] * P)           # [j, k, a, gm]
        w3q = w3q.transpose(2, 1, 0, 3)                 # [a, k, j, gm]
        w3q = w3q.reshape(A, P, Q * GR[3] * P)
        shared[f"{t}w3q"] = np.ascontiguousarray(w3q.astype(E4M3))
        shared[f"{t}b3"] = _tile_bias(b3)

    shared["scales"] = np.ascontiguousarray(
        np.broadcast_to(np.array([cal["o"][1], cal["d"][1]], np.float32),
                        (P, 2)))

    obsT = np.asarray(inputs["obs"], np.float32).T.astype(F16)    # [256, 4096]
    dltT = np.asarray(inputs["deltas"], np.float32).T.astype(F16)

    in_maps = []
    for c in range(NCORES):
        sl = slice(c * BPC, (c + 1) * BPC)
        m = dict(shared)
        m["xo"] = np.ascontiguousarray(
            obsT[:, sl].reshape(2, P, BPC).transpose(1, 0, 2).reshape(P, 2 * BPC))
        m["xd"] = np.ascontiguousarray(
            dltT[:, sl].reshape(2, P, BPC).transpose(1, 0, 2).reshape(P, 2 * BPC))
        in_maps.append(m)
    return in_maps


_PROGRAM = None


def kernel(**inputs):
    global _PROGRAM, LAST_RESULTS
    from concourse.bass_utils import run_bass_kernel_spmd

    if _PROGRAM is None:
        _PROGRAM = _build_program()
    in_maps = _prep_inputs(inputs)
    res = run_bass_kernel_spmd(_PROGRAM, in_maps, list(range(NCORES)))
    LAST_RESULTS = res
    out = np.empty((B, A), np.float32)
    for c in range(NCORES):
        out[c * BPC:(c + 1) * BPC] = res.results[c]["pred"].astype(
            np.float32).T
    return out
